# revision 12
# baseline (speedup 1.0000x reference)
"""EdgeEmbedding forward on 8 Trainium2 NeuronCores.

Computation (see reference):
    type_attr_sum[t] = sum_{j: attr_seg_ids[j]==t} attr_table[flat_attr_ids[j]]
    combined[t]      = edge_type_embedding[t] + type_attr_sum[t]        # [1000, 256]
    out[i]           = combined[data[i]]                                # [1M, 256]

Distribution / algorithm:
  Stage 1 (segment sum): the 50K ragged attr references are sharded across
  the 8 cores by attr-table row range.  Within a core the references are
  bucketed by seg>>7 (8 buckets of 1024 padded slots); each bucket is
  gathered with gpsimd.dma_gather, cast bf16, and reduced with one-hot PE
  matmuls into that bucket's 128-seg PSUM tile.  Each 128-seg chunk is
  AllReduced separately (pipelined across chunks); edge_type_embedding is
  folded in on core 0 only (its input; zeros elsewhere), yielding the
  combined table chunk-by-chunk.

  Stage 2 (edge gather): edges are sharded across cores (125K each) and,
  on the host, stably bucketed by type>>7 into 8 chunks padded to 16384
  rows.  The device holds each combined chunk in SBUF as fp16 and emits
  every output row with a one-hot matmul: oh[t_local, e] =
  (t_local == dval[e]) built on DVE from a uint8 dval tile (host
  pre-replicated across partitions), then PSUM[e, :] = oh.T @ chi_chunk.
  ACT evacuates PSUM in 4-tile batches, 2MB HWDGE DMAs write HBM.  No HBM
  gather reads - the only bulk HBM traffic is the output write plus a
  1-byte-per-edge index stream.  The host inverse-permutes rows on
  unshard (order within a chunk is preserved, pads dropped).
"""
import os
import sys

sys.path.insert(0, "/opt/trn_rl_repo")

import numpy as np

import concourse.bass as bass
import concourse.bacc as bacc
import concourse.mybir as mybir
from concourse.tile import TileContext
from concourse.bass_utils import run_bass_kernel_spmd

# ---- problem constants (hardcoded per harness contract) ----
N = 1_000_000
D = 256
NSEG = 1000
NSEG_PAD = 1024
ATTR_NUM = 200_000
NCORES = 8
ATTR_PER_CORE = ATTR_NUM // NCORES      # 25_000 table rows per core
E = N // NCORES                         # 125_000 edges per core

NCHUNK = 8            # type chunks of 128
L = 16_384            # padded edge slots per chunk (6.5 sigma above 15625 mean)
SEG = 4096            # edges per one-hot build granule
SEGS_PER_CHUNK = L // SEG               # 4
TILES_PER_SEG = SEG // 128              # 32
GROUP = 32            # 128-edge tiles per output DMA (2 MB fp16)
GROUPS_PER_SEG = TILES_PER_SEG // GROUP  # 1

S1_BUCKETS = 8        # seg chunks of 128
S1_SLOTS_PER_BUCKET = 1024              # 8.7 sigma above 781 mean
S1_KT_PER_BUCKET = S1_SLOTS_PER_BUCKET // 128   # 8
S1_SLOTS = S1_BUCKETS * S1_SLOTS_PER_BUCKET     # 8192

_cached = {}


def _build_program():
    if "nc" in _cached:
        return _cached["nc"]
    nc = bacc.Bacc("TRN2", target_bir_lowering=False, debug=False, num_devices=NCORES)

    f32 = mybir.dt.float32
    bf16 = mybir.dt.bfloat16
    fp16 = mybir.dt.float16
    u8 = mybir.dt.uint8

    attr_shard = nc.dram_tensor("attr_shard", [ATTR_PER_CORE, D], f32, kind="ExternalInput")
    edge_emb = nc.dram_tensor("edge_emb", [NSEG_PAD, D], f32, kind="ExternalInput")
    aidx = nc.dram_tensor("aidx", [128, S1_SLOTS // 16], mybir.dt.int16, kind="ExternalInput")
    asegf = nc.dram_tensor("asegf", [128, S1_BUCKETS * S1_KT_PER_BUCKET], bf16, kind="ExternalInput")
    iotaf = nc.dram_tensor("iotaf", [128, 128], bf16, kind="ExternalInput")
    iotap = nc.dram_tensor("iotap", [128, 1], u8, kind="ExternalInput")
    dval = nc.dram_tensor("dval", [128, NCHUNK * L], u8, kind="ExternalInput")
    out_dev = nc.dram_tensor("out_dev", [NCHUNK * L, D], fp16, kind="ExternalOutput")

    # per-chunk collective buffers: separate tensors so the tile framework
    # sees no false WAR/WAW between successive AllReduces (they pipeline)
    ar_ins = [nc.dram_tensor(f"ar_in{b}", [128, D], f32) for b in range(NCHUNK)]
    ar_outs = [nc.dram_tensor(f"ar_out{b}", [128, D], f32) for b in range(NCHUNK)]

    with TileContext(nc) as tc:
        with (
            tc.tile_pool(name="misc", bufs=1) as misc,
            tc.tile_pool(name="s1a", bufs=2) as s1a,
            tc.tile_pool(name="s1oh", bufs=2) as s1oh,
            tc.tile_pool(name="s1ps", bufs=2, space="PSUM") as s1ps,
            tc.tile_pool(name="s1out", bufs=2) as s1out,
            tc.tile_pool(name="chip", bufs=NCHUNK) as chip,
            tc.tile_pool(name="s2dvr", bufs=3) as s2dvr,
            tc.tile_pool(name="s2oh", bufs=3) as s2oh,
            tc.tile_pool(name="s2ps", bufs=3, space="PSUM") as s2ps,
            tc.tile_pool(name="s2st", bufs=3) as s2st,
        ):
            # ---- prologue: constants / index tables (SP ring) ----
            aidx_t = misc.tile([128, S1_SLOTS // 16], mybir.dt.int16)
            nc.sync.dma_start(out=aidx_t[:, :], in_=aidx.ap())
            asegf_t = misc.tile([128, S1_BUCKETS * S1_KT_PER_BUCKET], bf16)
            nc.sync.dma_start(out=asegf_t[:, :], in_=asegf.ap())
            iotaf_t = misc.tile([128, 128], bf16)
            nc.sync.dma_start(out=iotaf_t[:, :], in_=iotaf.ap())
            iotap_t = misc.tile([128, 1], u8)
            nc.sync.dma_start(out=iotap_t[:, :], in_=iotap.ap())

            # ======== stage 1 for ALL buckets first: emission order is the
            # scheduler's priority, so the gather -> segment-sum -> AllReduce
            # -> chi ladder always wins the per-engine ready-heap and stage-2
            # work below fills the idle slots around it. ========
            chis = []
            for b in range(NCHUNK):
                # ================= stage 1, seg bucket b =================
                atile = s1a.tile([128, S1_KT_PER_BUCKET, D], f32, tag="atile", name=f"atile{b}")
                nc.gpsimd.dma_gather(
                    out_ap=atile[:, :, :],
                    in_ap=attr_shard.ap(),
                    idxs_ap=aidx_t[:, b * (S1_SLOTS_PER_BUCKET // 16):(b + 1) * (S1_SLOTS_PER_BUCKET // 16)],
                    num_idxs=S1_SLOTS_PER_BUCKET,
                    num_idxs_reg=S1_SLOTS_PER_BUCKET,
                    elem_size=D,
                    single_packet=False,
                )
                abf = s1a.tile([128, S1_KT_PER_BUCKET, D], bf16, tag="abf", name=f"abf{b}")
                nc.vector.tensor_copy(abf[:, :, :], atile[:, :, :])

                ps1 = s1ps.tile([128, D], f32, tag="ps1", name=f"ps1_{b}")
                for c in range(S1_KT_PER_BUCKET):
                    kt = b * S1_KT_PER_BUCKET + c
                    oh1 = s1oh.tile([128, 128], bf16, tag="oh1", name=f"oh1_{b}_{c}")
                    nc.vector.tensor_tensor(
                        oh1[:, :],
                        asegf_t[:, kt:kt + 1].broadcast_to((128, 128)),
                        iotaf_t[:, :],
                        op=mybir.AluOpType.is_equal,
                    )
                    nc.tensor.matmul(
                        ps1[:, :], oh1[:, :], abf[:, c, :],
                        start=(c == 0), stop=(c == S1_KT_PER_BUCKET - 1),
                    )
                part = s1out.tile([128, D], f32, tag="part", name=f"part{b}")
                nc.scalar.copy(part[:, :], ps1[:, :])
                embt = s1out.tile([128, D], f32, tag="embt", name=f"embt{b}")
                nc.scalar.dma_start(out=embt[:, :], in_=edge_emb.ap()[b * 128:(b + 1) * 128, :])
                nc.vector.tensor_add(part[:, :], part[:, :], embt[:, :])
                nc.sync.dma_start(out=ar_ins[b].ap(), in_=part[:, :])

                nc.gpsimd.collective_compute(
                    "AllReduce", mybir.AluOpType.add,
                    replica_groups=[list(range(NCORES))],
                    ins=[ar_ins[b].ap().opt()],
                    outs=[ar_outs[b].ap().opt()],
                )

                # combined chunk: load f32, cast fp16 on ACT right before use
                ctmp = s1out.tile([128, D], f32, tag="ctmp", name=f"ctmp{b}")
                nc.scalar.dma_start(out=ctmp[:, :], in_=ar_outs[b].ap())
                chi = chip.tile([128, D], fp16, tag="chi", name=f"chi{b}")
                nc.scalar.copy(chi[:, :], ctmp[:, :])
                chis.append(chi)

            # ======== stage 2: all chunks ========
            for b in range(NCHUNK):
                chi = chis[b]
                for s in range(SEGS_PER_CHUNK):
                    off = b * L + s * SEG
                    dvr = s2dvr.tile([128, SEG], u8, tag="dvr", name=f"dvr{b}_{s}")
                    nc.scalar.dma_start(out=dvr[:, :], in_=dval.ap()[:, off:off + SEG])
                    oh2 = s2oh.tile([128, SEG], fp16, tag="oh2", name=f"oh2_{b}_{s}")
                    nc.vector.tensor_tensor(
                        oh2[:, :],
                        iotap_t[:, 0:1].broadcast_to((128, SEG)),
                        dvr[:, :],
                        op=mybir.AluOpType.is_equal,
                    )
                    for g in range(GROUPS_PER_SEG):
                        st = s2st.tile([128, GROUP, D], fp16, tag="st", name=f"st{b}_{s}_{g}")
                        for j in range(GROUP // 4):
                            pp = s2ps.tile([128, 4, D], f32, tag="pp", name=f"pp{b}_{s}_{g}_{j}")
                            for h in range(4):
                                u = g * GROUP + j * 4 + h
                                nc.tensor.matmul(
                                    pp[:, h, :],
                                    oh2[:, u * 128:(u + 1) * 128],
                                    chi[:, :],
                                    start=True, stop=True,
                                )
                            if j % 4 == 3:
                                nc.vector.tensor_copy(st[:, j * 4:j * 4 + 4, :], pp[:, :, :])
                            else:
                                nc.scalar.copy(st[:, j * 4:j * 4 + 4, :], pp[:, :, :])
                        row0 = off + g * GROUP * 128
                        dst = bass.AP(out_dev, row0 * D, [[D, 128], [128 * D, GROUP], [1, D]])
                        nc.sync.dma_start(out=dst, in_=st[:, :, :])

    nc.compile()
    _cached["nc"] = nc
    return nc


def _wrap16(arr):
    """Position j -> [j%16, j//16] layout expected by dma_gather idx tensors."""
    assert arr.shape[0] % 16 == 0
    return arr.reshape(arr.shape[0] // 16, 16).T


def _prep_in_maps(data, attr_table, edge_type_embedding, flat_attr_ids, attr_seg_ids):
    import ml_dtypes
    bf16 = ml_dtypes.bfloat16

    ids = np.asarray(flat_attr_ids).astype(np.int64)
    segs = np.asarray(attr_seg_ids).astype(np.int64)
    data = np.asarray(data).astype(np.int64)
    attr_table = np.ascontiguousarray(np.asarray(attr_table, dtype=np.float32))
    edge_emb = np.zeros((NSEG_PAD, D), np.float32)
    edge_emb[:NSEG] = np.asarray(edge_type_embedding, dtype=np.float32)
    edge_emb_zero = np.zeros((NSEG_PAD, D), np.float32)

    iotaf = np.tile(np.arange(128, dtype=np.float32)[None, :], (128, 1)).astype(bf16)
    iotap = np.arange(128, dtype=np.uint8)[:, None]

    in_maps = []
    dev_idx = []
    for k in range(NCORES):
        # ---- stage 1: this core's attr references, bucketed by seg>>7 ----
        lo_id, hi_id = k * ATTR_PER_CORE, (k + 1) * ATTR_PER_CORE
        sel = (ids >= lo_id) & (ids < hi_id)
        ids_k = ids[sel] - lo_id
        segs_k = segs[sel]
        aid = np.zeros(S1_SLOTS, np.int64)
        aseg = np.full(S1_SLOTS, -1.0, np.float32)
        for b in range(S1_BUCKETS):
            m = (segs_k >> 7) == b
            nb = int(m.sum())
            assert nb <= S1_SLOTS_PER_BUCKET, f"s1 bucket overflow core {k} bucket {b}: {nb}"
            base = b * S1_SLOTS_PER_BUCKET
            aid[base:base + nb] = ids_k[m]
            aseg[base:base + nb] = (segs_k[m] - 128 * b).astype(np.float32)
        # per-bucket 16-wrap, concatenated along columns
        aidx16 = np.concatenate(
            [_wrap16(aid[b * S1_SLOTS_PER_BUCKET:(b + 1) * S1_SLOTS_PER_BUCKET])
             for b in range(S1_BUCKETS)], axis=1).astype(np.int16)
        aidx16 = np.tile(aidx16, (8, 1))                       # [128, 512]
        # slot (b, c, p) -> asegf[p, b*8+c]
        asegf_arr = np.ascontiguousarray(
            aseg.reshape(S1_BUCKETS * S1_KT_PER_BUCKET, 128).T.astype(bf16))

        # ---- stage 2: bucket edges by type>>7, stable, padded to L ----
        shard = data[k * E:(k + 1) * E]
        cid = shard >> 7
        dv = np.full(NCHUNK * L, 255, np.uint8)   # 255 = pad (never matches 0..127)
        didx = np.empty(E, np.int64)
        for c in range(NCHUNK):
            pos = np.nonzero(cid == c)[0]
            ncnt = pos.shape[0]
            assert ncnt <= L, f"edge chunk overflow core {k} chunk {c}: {ncnt}"
            dv[c * L:c * L + ncnt] = (shard[pos] - 128 * c).astype(np.uint8)
            didx[pos] = c * L + np.arange(ncnt)
        dev_idx.append(didx)

        in_maps.append({
            "attr_shard": np.ascontiguousarray(attr_table[lo_id:hi_id]),
            "edge_emb": edge_emb if k == 0 else edge_emb_zero,
            "aidx": np.ascontiguousarray(aidx16),
            "asegf": asegf_arr,
            "iotaf": iotaf,
            "iotap": iotap,
            "dval": np.ascontiguousarray(np.broadcast_to(dv[None, :], (128, NCHUNK * L))),
        })
    return in_maps, dev_idx


def run(inputs, trace=False, trace_cores=None):
    nc = _build_program()
    in_maps, dev_idx = _prep_in_maps(**inputs)
    kwargs = {}
    if trace:
        kwargs = dict(trace=True)
        if trace_cores is not None:
            kwargs["trace_cores"] = trace_cores
    res = run_bass_kernel_spmd(nc, in_maps, core_ids=list(range(NCORES)), **kwargs)
    outp = np.empty((N, D), np.float32)
    for k in range(NCORES):
        outp[k * E:(k + 1) * E] = res.results[k]["out_dev"][dev_idx[k]].astype(np.float32)
    return outp, res


def kernel(**inputs) -> np.ndarray:
    outp, _ = run(inputs, trace=False)
    return outp


# revision 15
# speedup vs baseline: 1.0686x; 1.0686x over previous
"""EdgeEmbedding forward on 8 Trainium2 NeuronCores.

Computation (see reference):
    type_attr_sum[t] = sum_{j: attr_seg_ids[j]==t} attr_table[flat_attr_ids[j]]
    combined[t]      = edge_type_embedding[t] + type_attr_sum[t]        # [1000, 256]
    out[i]           = combined[data[i]]                                # [1M, 256]

Distribution / algorithm:
  Stage 1 (segment sum): the 50K ragged attr references are sharded across
  the 8 cores by attr-table row range.  Within a core the references are
  bucketed by seg>>7 (8 buckets of 1024 padded slots); each bucket is
  gathered with gpsimd.dma_gather, cast bf16, and reduced with one-hot PE
  matmuls into that bucket's 128-seg PSUM tile.  Each 128-seg chunk is
  AllReduced separately (pipelined across chunks); edge_type_embedding is
  folded in on core 0 only (its input; zeros elsewhere), yielding the
  combined table chunk-by-chunk.

  Stage 2 (edge gather): edges are sharded across cores (125K each) and,
  on the host, stably bucketed by type>>7 into 8 chunks padded to 16384
  rows.  The device holds each combined chunk in SBUF as fp16 and emits
  every output row with a one-hot matmul: oh[t_local, e] =
  (t_local == dval[e]) built on DVE from a uint8 dval tile (host
  pre-replicated across partitions), then PSUM[e, :] = oh.T @ chi_chunk.
  ACT evacuates PSUM in 4-tile batches, 2MB HWDGE DMAs write HBM.  No HBM
  gather reads - the only bulk HBM traffic is the output write plus a
  1-byte-per-edge index stream.  The host inverse-permutes rows on
  unshard (order within a chunk is preserved, pads dropped).
"""
import os
import sys

sys.path.insert(0, "/opt/trn_rl_repo")

import numpy as np

import concourse.bass as bass
import concourse.bacc as bacc
import concourse.mybir as mybir
from concourse.tile import TileContext
from concourse.bass_utils import run_bass_kernel_spmd

# ---- problem constants (hardcoded per harness contract) ----
N = 1_000_000
D = 256
NSEG = 1000
NSEG_PAD = 1024
ATTR_NUM = 200_000
NCORES = 8
ATTR_PER_CORE = ATTR_NUM // NCORES      # 25_000 table rows per core
E = N // NCORES                         # 125_000 edges per core

NCHUNK = 8            # type chunks of 128
L = 16_384            # padded edge slots per chunk (6.5 sigma above 15625 mean)
SEG = 4096            # edges per one-hot build granule
SEGS_PER_CHUNK = L // SEG               # 4
TILES_PER_SEG = SEG // 128              # 32
GROUP = 32            # 128-edge tiles per output DMA (2 MB fp16)
GROUPS_PER_SEG = TILES_PER_SEG // GROUP  # 1

S1_BUCKETS = 8        # seg chunks of 128
S1_SLOTS_PER_BUCKET = 1024              # 8.7 sigma above 781 mean
S1_KT_PER_BUCKET = S1_SLOTS_PER_BUCKET // 128   # 8
S1_SLOTS = S1_BUCKETS * S1_SLOTS_PER_BUCKET     # 8192

_cached = {}


def _build_program():
    if "nc" in _cached:
        return _cached["nc"]
    # The tile scheduler plans with the hw-spec cost model; its stock SWDGE
    # per-descriptor estimate (0.34ns) is ~25x below measured dma_gather HW
    # behavior (~9ns/desc), which makes it front-load all gathers ahead of
    # the first AllReduce dispatch and serialize the collective chain late.
    # Patch the constant to the measured value for OUR build only, restore
    # afterwards.
    from concourse import hw_specs
    _orig_swdge = hw_specs.TRN2Spec.SWDGE_NS_PER_DESCRIPTOR
    hw_specs.TRN2Spec.SWDGE_NS_PER_DESCRIPTOR = 9.0
    try:
        nc = _build_program_inner()
    finally:
        hw_specs.TRN2Spec.SWDGE_NS_PER_DESCRIPTOR = _orig_swdge
    _cached["nc"] = nc
    return nc


def _build_program_inner():
    nc = bacc.Bacc("TRN2", target_bir_lowering=False, debug=False, num_devices=NCORES)

    f32 = mybir.dt.float32
    bf16 = mybir.dt.bfloat16
    fp16 = mybir.dt.float16
    u8 = mybir.dt.uint8

    attr_shard = nc.dram_tensor("attr_shard", [ATTR_PER_CORE, D], f32, kind="ExternalInput")
    edge_emb = nc.dram_tensor("edge_emb", [NSEG_PAD, D], f32, kind="ExternalInput")
    aidx = nc.dram_tensor("aidx", [128, S1_SLOTS // 16], mybir.dt.int16, kind="ExternalInput")
    asegf = nc.dram_tensor("asegf", [128, S1_BUCKETS * S1_KT_PER_BUCKET], bf16, kind="ExternalInput")
    iotaf = nc.dram_tensor("iotaf", [128, 128], bf16, kind="ExternalInput")
    iotap = nc.dram_tensor("iotap", [128, 1], u8, kind="ExternalInput")
    dval = nc.dram_tensor("dval", [128, NCHUNK * L], u8, kind="ExternalInput")
    out_dev = nc.dram_tensor("out_dev", [NCHUNK * L, D], fp16, kind="ExternalOutput")

    # per-chunk collective buffers: separate tensors so the tile framework
    # sees no false WAR/WAW between successive AllReduces (they pipeline)
    ar_ins = [nc.dram_tensor(f"ar_in{b}", [128, D], f32) for b in range(NCHUNK)]
    ar_outs = [nc.dram_tensor(f"ar_out{b}", [128, D], f32) for b in range(NCHUNK)]

    with TileContext(nc) as tc:
        with (
            tc.tile_pool(name="misc", bufs=1) as misc,
            tc.tile_pool(name="s1a", bufs=2) as s1a,
            tc.tile_pool(name="s1oh", bufs=2) as s1oh,
            tc.tile_pool(name="s1ps", bufs=2, space="PSUM") as s1ps,
            tc.tile_pool(name="s1out", bufs=2) as s1out,
            tc.tile_pool(name="chip", bufs=NCHUNK) as chip,
            tc.tile_pool(name="s2dvr", bufs=3) as s2dvr,
            tc.tile_pool(name="s2oh", bufs=3) as s2oh,
            tc.tile_pool(name="s2ps", bufs=3, space="PSUM") as s2ps,
            tc.tile_pool(name="s2st", bufs=4) as s2st,
        ):
            # ---- prologue: constants / index tables (SP ring) ----
            aidx_t = misc.tile([128, S1_SLOTS // 16], mybir.dt.int16)
            nc.sync.dma_start(out=aidx_t[:, :], in_=aidx.ap())
            asegf_t = misc.tile([128, S1_BUCKETS * S1_KT_PER_BUCKET], bf16)
            nc.sync.dma_start(out=asegf_t[:, :], in_=asegf.ap())
            iotaf_t = misc.tile([128, 128], bf16)
            nc.sync.dma_start(out=iotaf_t[:, :], in_=iotaf.ap())
            iotap_t = misc.tile([128, 1], u8)
            nc.sync.dma_start(out=iotap_t[:, :], in_=iotap.ap())

            # ======== stage 1 for ALL buckets first: emission order is the
            # scheduler's priority, so the gather -> segment-sum -> AllReduce
            # -> chi ladder always wins the per-engine ready-heap and stage-2
            # work below fills the idle slots around it. ========
            chis = []
            for b in range(NCHUNK):
                # ================= stage 1, seg bucket b =================
                atile = s1a.tile([128, S1_KT_PER_BUCKET, D], f32, tag="atile", name=f"atile{b}")
                nc.gpsimd.dma_gather(
                    out_ap=atile[:, :, :],
                    in_ap=attr_shard.ap(),
                    idxs_ap=aidx_t[:, b * (S1_SLOTS_PER_BUCKET // 16):(b + 1) * (S1_SLOTS_PER_BUCKET // 16)],
                    num_idxs=S1_SLOTS_PER_BUCKET,
                    num_idxs_reg=S1_SLOTS_PER_BUCKET,
                    elem_size=D,
                    single_packet=False,
                )
                abf = s1a.tile([128, S1_KT_PER_BUCKET, D], bf16, tag="abf", name=f"abf{b}")
                nc.vector.tensor_copy(abf[:, :, :], atile[:, :, :])

                ps1 = s1ps.tile([128, D], f32, tag="ps1", name=f"ps1_{b}")
                for c in range(S1_KT_PER_BUCKET):
                    kt = b * S1_KT_PER_BUCKET + c
                    oh1 = s1oh.tile([128, 128], bf16, tag="oh1", name=f"oh1_{b}_{c}")
                    nc.vector.tensor_tensor(
                        oh1[:, :],
                        asegf_t[:, kt:kt + 1].broadcast_to((128, 128)),
                        iotaf_t[:, :],
                        op=mybir.AluOpType.is_equal,
                    )
                    nc.tensor.matmul(
                        ps1[:, :], oh1[:, :], abf[:, c, :],
                        start=(c == 0), stop=(c == S1_KT_PER_BUCKET - 1),
                    )
                part = s1out.tile([128, D], f32, tag="part", name=f"part{b}")
                nc.scalar.copy(part[:, :], ps1[:, :])
                embt = s1out.tile([128, D], f32, tag="embt", name=f"embt{b}")
                nc.scalar.dma_start(out=embt[:, :], in_=edge_emb.ap()[b * 128:(b + 1) * 128, :])
                nc.vector.tensor_add(part[:, :], part[:, :], embt[:, :])
                nc.sync.dma_start(out=ar_ins[b].ap(), in_=part[:, :])

                nc.gpsimd.collective_compute(
                    "AllReduce", mybir.AluOpType.add,
                    replica_groups=[list(range(NCORES))],
                    ins=[ar_ins[b].ap().opt()],
                    outs=[ar_outs[b].ap().opt()],
                )

                # combined chunk: load f32, cast fp16 on ACT right before use
                ctmp = s1out.tile([128, D], f32, tag="ctmp", name=f"ctmp{b}")
                nc.scalar.dma_start(out=ctmp[:, :], in_=ar_outs[b].ap())
                chi = chip.tile([128, D], fp16, tag="chi", name=f"chi{b}")
                nc.scalar.copy(chi[:, :], ctmp[:, :])
                chis.append(chi)

            # ======== stage 2: all chunks ========
            for b in range(NCHUNK):
                chi = chis[b]
                for s in range(SEGS_PER_CHUNK):
                    off = b * L + s * SEG
                    dvr = s2dvr.tile([128, SEG], u8, tag="dvr", name=f"dvr{b}_{s}")
                    nc.scalar.dma_start(out=dvr[:, :], in_=dval.ap()[:, off:off + SEG])
                    oh2 = s2oh.tile([128, SEG], fp16, tag="oh2", name=f"oh2_{b}_{s}")
                    nc.vector.tensor_tensor(
                        oh2[:, :],
                        iotap_t[:, 0:1].broadcast_to((128, SEG)),
                        dvr[:, :],
                        op=mybir.AluOpType.is_equal,
                    )
                    for g in range(GROUPS_PER_SEG):
                        st = s2st.tile([128, GROUP, D], fp16, tag="st", name=f"st{b}_{s}_{g}")
                        for j in range(GROUP // 4):
                            pp = s2ps.tile([128, 4, D], f32, tag="pp", name=f"pp{b}_{s}_{g}_{j}")
                            for h in range(4):
                                u = g * GROUP + j * 4 + h
                                nc.tensor.matmul(
                                    pp[:, h, :],
                                    oh2[:, u * 128:(u + 1) * 128],
                                    chi[:, :],
                                    start=True, stop=True,
                                )
                            if j % 4 == 3:
                                nc.vector.tensor_copy(st[:, j * 4:j * 4 + 4, :], pp[:, :, :])
                            else:
                                nc.scalar.copy(st[:, j * 4:j * 4 + 4, :], pp[:, :, :])
                        row0 = off + g * GROUP * 128
                        dst = bass.AP(out_dev, row0 * D, [[D, 128], [128 * D, GROUP], [1, D]])
                        nc.sync.dma_start(out=dst, in_=st[:, :, :])

    nc.compile()
    return nc


def _wrap16(arr):
    """Position j -> [j%16, j//16] layout expected by dma_gather idx tensors."""
    assert arr.shape[0] % 16 == 0
    return arr.reshape(arr.shape[0] // 16, 16).T


def _prep_in_maps(data, attr_table, edge_type_embedding, flat_attr_ids, attr_seg_ids):
    import ml_dtypes
    bf16 = ml_dtypes.bfloat16

    ids = np.asarray(flat_attr_ids).astype(np.int64)
    segs = np.asarray(attr_seg_ids).astype(np.int64)
    data = np.asarray(data).astype(np.int64)
    attr_table = np.ascontiguousarray(np.asarray(attr_table, dtype=np.float32))
    edge_emb = np.zeros((NSEG_PAD, D), np.float32)
    edge_emb[:NSEG] = np.asarray(edge_type_embedding, dtype=np.float32)
    edge_emb_zero = np.zeros((NSEG_PAD, D), np.float32)

    iotaf = np.tile(np.arange(128, dtype=np.float32)[None, :], (128, 1)).astype(bf16)
    iotap = np.arange(128, dtype=np.uint8)[:, None]

    in_maps = []
    dev_idx = []
    for k in range(NCORES):
        # ---- stage 1: this core's attr references, bucketed by seg>>7 ----
        lo_id, hi_id = k * ATTR_PER_CORE, (k + 1) * ATTR_PER_CORE
        sel = (ids >= lo_id) & (ids < hi_id)
        ids_k = ids[sel] - lo_id
        segs_k = segs[sel]
        aid = np.zeros(S1_SLOTS, np.int64)
        aseg = np.full(S1_SLOTS, -1.0, np.float32)
        for b in range(S1_BUCKETS):
            m = (segs_k >> 7) == b
            nb = int(m.sum())
            assert nb <= S1_SLOTS_PER_BUCKET, f"s1 bucket overflow core {k} bucket {b}: {nb}"
            base = b * S1_SLOTS_PER_BUCKET
            aid[base:base + nb] = ids_k[m]
            aseg[base:base + nb] = (segs_k[m] - 128 * b).astype(np.float32)
        # per-bucket 16-wrap, concatenated along columns
        aidx16 = np.concatenate(
            [_wrap16(aid[b * S1_SLOTS_PER_BUCKET:(b + 1) * S1_SLOTS_PER_BUCKET])
             for b in range(S1_BUCKETS)], axis=1).astype(np.int16)
        aidx16 = np.tile(aidx16, (8, 1))                       # [128, 512]
        # slot (b, c, p) -> asegf[p, b*8+c]
        asegf_arr = np.ascontiguousarray(
            aseg.reshape(S1_BUCKETS * S1_KT_PER_BUCKET, 128).T.astype(bf16))

        # ---- stage 2: bucket edges by type>>7, stable, padded to L ----
        shard = data[k * E:(k + 1) * E]
        cid = shard >> 7
        dv = np.full(NCHUNK * L, 255, np.uint8)   # 255 = pad (never matches 0..127)
        didx = np.empty(E, np.int64)
        for c in range(NCHUNK):
            pos = np.nonzero(cid == c)[0]
            ncnt = pos.shape[0]
            assert ncnt <= L, f"edge chunk overflow core {k} chunk {c}: {ncnt}"
            dv[c * L:c * L + ncnt] = (shard[pos] - 128 * c).astype(np.uint8)
            didx[pos] = c * L + np.arange(ncnt)
        dev_idx.append(didx)

        in_maps.append({
            "attr_shard": np.ascontiguousarray(attr_table[lo_id:hi_id]),
            "edge_emb": edge_emb if k == 0 else edge_emb_zero,
            "aidx": np.ascontiguousarray(aidx16),
            "asegf": asegf_arr,
            "iotaf": iotaf,
            "iotap": iotap,
            "dval": np.ascontiguousarray(np.broadcast_to(dv[None, :], (128, NCHUNK * L))),
        })
    return in_maps, dev_idx


def run(inputs, trace=False, trace_cores=None):
    nc = _build_program()
    in_maps, dev_idx = _prep_in_maps(**inputs)
    kwargs = {}
    if trace:
        kwargs = dict(trace=True)
        if trace_cores is not None:
            kwargs["trace_cores"] = trace_cores
    res = run_bass_kernel_spmd(nc, in_maps, core_ids=list(range(NCORES)), **kwargs)
    outp = np.empty((N, D), np.float32)
    for k in range(NCORES):
        outp[k * E:(k + 1) * E] = res.results[k]["out_dev"][dev_idx[k]].astype(np.float32)
    return outp, res


def kernel(**inputs) -> np.ndarray:
    outp, _ = run(inputs, trace=False)
    return outp
